# revision 4
# baseline (speedup 1.0000x reference)
"""Trainium2 Bass kernel for DFTB Slater-Koster table interpolation.

Problem (see reference): for 2M edges, linearly interpolate 16 hopping and 16
overlap integrals from per-bond-type tables [10, 16, 512] at distance rij;
for 500K nodes, look up onsite energies [4, 4] by atom type.

Strategy (8 NeuronCores, data-parallel over edges/nodes):
  - Host packs a "pair table" [5120, 64] f32 where row (b*512+g) holds
    [hop[b,:,g], ovl[b,:,g], hop[b,:,g+1]-hop[b,:,g], ovl[b,:,g+1]-ovl[b,:,g]]
    so one 256B SWDGE dma_gather descriptor per edge fetches everything needed.
  - Device computes i0 = floor((rij-X0)/DX) (exact floor, robust to the
    engine's f32->int rounding mode) and the combined table row index
    idx = etype*512 + i0 as int16, gathers 256B/edge with dma_gather, then
    interpolates y0 + frac*delta on the vector engine with a per-edge
    broadcast of frac.
  - Edges are processed in gather tiles of NI=8192 (the SWDGE descriptor ring
    holds 1024 descriptors per engine; NI/16 must stay below that).
    dma_gather places edge j of a tile at SBUF (partition j%128, group
    j//128) and reads its index from (partition j%16, column j//16); the host
    pre-packs rij into both wrap-16 (index path) and wrap-128 (frac path)
    layouts so every device DMA is a plain linear access pattern, and outputs
    are written partition-major so write descriptors are 4KB-contiguous.
  - Node onsite lookup: with only 4 atom types, evaluate the cubic polynomial
    through the 4 table values (exact interpolation at a=0..3) instead of a
    gather.
"""
import numpy as np

import concourse.bass as bass
import concourse.bacc as bacc
import concourse.mybir as mybir
import concourse.tile as tile
from concourse import library_config
from concourse.bass_utils import run_bass_kernel_spmd

# ---- problem constants (hardcoded; must match the reference) ----
NGRID = 512
X0 = 1.0
XMAX = 10.0
DX = (XMAX - X0) / (NGRID - 1)
NB = 10
M = 16
NROWS = NB * NGRID          # 5120 pair-table rows
ES = 4 * M                  # 64 f32 = 256B per row
N_ATOM_TYPES = 4
N_ONSITE = 4

NCORES = 8
E_TOTAL = 2_000_000
N_TOTAL = 500_000

# per-core tiling
E_SHARD = E_TOTAL // NCORES          # 250_000
NI = 8192                            # edges per gather tile
T = 32                               # gather tiles per core
E_PAD = T * NI                       # 262_144
G = NI // 128                        # 64 groups per tile
D = NI // 16                         # 512 idx cols per tile
GT = T * G                           # 2048
DT = T * D                           # 16384
N_SHARD = N_TOTAL // NCORES          # 62_500
KN = 512                             # node cols per partition
N_PAD = 128 * KN                     # 65_536

LAST_RESULTS = None  # BassKernelResults of the most recent run (for profiling)


def _build_pair_table(hop: np.ndarray, ovl: np.ndarray) -> np.ndarray:
    ptab = np.empty((NB, NGRID, ES), np.float32)
    h = np.ascontiguousarray(hop.transpose(0, 2, 1))  # [NB, G, M]
    o = np.ascontiguousarray(ovl.transpose(0, 2, 1))
    ptab[:, :, 0:16] = h
    ptab[:, :, 16:32] = o
    ptab[:, :-1, 32:48] = h[:, 1:] - h[:, :-1]
    ptab[:, -1, 32:48] = 0.0
    ptab[:, :-1, 48:64] = o[:, 1:] - o[:, :-1]
    ptab[:, -1, 48:64] = 0.0
    return ptab.reshape(NROWS, ES)


def _node_poly_coeffs(onsiteE: np.ndarray) -> np.ndarray:
    """Cubic through onsiteE[a, j] at a = 0..3 (exact). [4 coefs, 4 feats]."""
    A = np.vander(np.arange(4.0), 4, increasing=False)
    return np.linalg.solve(A.astype(np.float64), onsiteE.astype(np.float64))


def _emit_floor(nc, pool, src_f32, parts, width, tagp):
    """Exact floor of src (values >= -0.5) as an f32 tile.
    i0f = float(int_cast(src)); if i0f > src: i0f -= 1. Correct whether the
    engine's f32->int cast truncates or rounds to nearest."""
    f32, i32 = mybir.dt.float32, mybir.dt.int32
    ii = pool.tile([parts, width], i32, tag=f"{tagp}_ii")
    nc.vector.tensor_copy(out=ii[:], in_=src_f32)
    ff = pool.tile([parts, width], f32, tag=f"{tagp}_ff")
    nc.vector.tensor_copy(out=ff[:], in_=ii[:])
    gt = pool.tile([parts, width], f32, tag=f"{tagp}_gt")
    nc.vector.tensor_tensor(out=gt[:], in0=ff[:], in1=src_f32,
                            op=mybir.AluOpType.is_gt)
    nc.vector.tensor_tensor(out=ff[:], in0=ff[:], in1=gt[:],
                            op=mybir.AluOpType.subtract)
    return ff


def _build_program(coefs: np.ndarray):
    nc = bacc.Bacc("TRN2", target_bir_lowering=False, debug=False)
    f32, i32, i16 = mybir.dt.float32, mybir.dt.int32, mybir.dt.int16

    ptab = nc.dram_tensor("ptab", [NROWS, ES], f32, kind="ExternalInput")
    rij16 = nc.dram_tensor("rij16", [16, DT], f32, kind="ExternalInput")
    et16 = nc.dram_tensor("et16", [16, DT], i32, kind="ExternalInput")
    rij128 = nc.dram_tensor("rij128", [128, GT], f32, kind="ExternalInput")
    atype = nc.dram_tensor("atype", [128, KN], i32, kind="ExternalInput")
    hop_out = nc.dram_tensor("hop_out", [128, GT, M], f32, kind="ExternalOutput")
    ovl_out = nc.dram_tensor("ovl_out", [128, GT, M], f32, kind="ExternalOutput")
    node_out = nc.dram_tensor("node_out", [128, KN, 4], f32, kind="ExternalOutput")
    idx_dram = nc.dram_tensor("idx_scratch", [T, 16, D], i16, kind="Internal")

    inv_dx = 1.0 / DX

    with tile.TileContext(nc) as tc:
        with (
            tc.tile_pool(name="io", bufs=3) as io,
            tc.tile_pool(name="gat", bufs=3) as gat,
            tc.tile_pool(name="sm", bufs=3) as sm,
            tc.tile_pool(name="one", bufs=1) as one,
        ):
            nc.gpsimd.load_library(library_config.mlp)

            for t in range(T):
                # ---- index path, wrap-16 layout [16, D] ----
                rv = sm.tile([16, D], f32, tag="rv")
                nc.sync.dma_start(rv[:], rij16[:, t * D : (t + 1) * D])
                ev = sm.tile([16, D], i32, tag="ev")
                nc.sync.dma_start(ev[:], et16[:, t * D : (t + 1) * D])
                # t = (rij - X0) / DX   (in-place on rv)
                nc.vector.tensor_scalar(
                    out=rv[:], in0=rv[:], scalar1=X0, scalar2=inv_dx,
                    op0=mybir.AluOpType.subtract, op1=mybir.AluOpType.mult)
                i0f = _emit_floor(nc, sm, rv[:], 16, D, "ix")
                i0i = sm.tile([16, D], i32, tag="i0i")
                nc.vector.tensor_copy(out=i0i[:], in_=i0f[:])  # integral: exact
                # idx = (etype << 9) + i0   (in-place on ev)
                nc.vector.tensor_scalar(
                    out=ev[:], in0=ev[:], scalar1=9, scalar2=None,
                    op0=mybir.AluOpType.logical_shift_left)
                nc.vector.tensor_tensor(out=ev[:], in0=ev[:], in1=i0i[:],
                                        op=mybir.AluOpType.add)
                ix16 = sm.tile([16, D], i16, tag="ix16")
                nc.vector.tensor_copy(out=ix16[:], in_=ev[:])
                # replicate [16, D] -> [128, D] via DRAM roundtrip (the gather
                # engine reads indices from all 8 partition groups)
                nc.sync.dma_start(idx_dram[t], ix16[:])
                rep = io.tile([128, D], i16, tag="rep")
                rep_src = bass.AP(idx_dram, t * 16 * D, [[0, 8], [D, 16], [1, D]])
                nc.sync.dma_start(rep[:], rep_src)
                # ---- frac path, wrap-128 layout [128, G] ----
                rf = sm.tile([128, G], f32, tag="rf")
                nc.scalar.dma_start(rf[:], rij128[:, t * G : (t + 1) * G])
                nc.vector.tensor_scalar(
                    out=rf[:], in0=rf[:], scalar1=X0, scalar2=inv_dx,
                    op0=mybir.AluOpType.subtract, op1=mybir.AluOpType.mult)
                f0f = _emit_floor(nc, sm, rf[:], 128, G, "fr")
                frac = sm.tile([128, G], f32, tag="frac")
                nc.vector.tensor_tensor(out=frac[:], in0=rf[:], in1=f0f[:],
                                        op=mybir.AluOpType.subtract)
                # ---- gather: 256B per edge ----
                dst = gat.tile([128, G, ES], f32, tag="dst")
                nc.gpsimd.dma_gather(dst[:], ptab[:, :], rep[:], NI, NI, ES,
                                     single_packet=False)
                # ---- interpolate: out = y0 + frac * delta ----
                fb = frac[:].rearrange("p (g o) -> p g o", o=1).to_broadcast([128, G, M])
                oh = io.tile([128, G, M], f32, tag="oh")
                nc.vector.tensor_tensor(out=oh[:], in0=dst[:, :, 32:48], in1=fb,
                                        op=mybir.AluOpType.mult)
                nc.vector.tensor_tensor(out=oh[:], in0=oh[:], in1=dst[:, :, 0:16],
                                        op=mybir.AluOpType.add)
                oo = io.tile([128, G, M], f32, tag="oo")
                nc.vector.tensor_tensor(out=oo[:], in0=dst[:, :, 48:64], in1=fb,
                                        op=mybir.AluOpType.mult)
                nc.vector.tensor_tensor(out=oo[:], in0=oo[:], in1=dst[:, :, 16:32],
                                        op=mybir.AluOpType.add)
                nc.scalar.dma_start(hop_out[:, t * G : (t + 1) * G, :], oh[:])
                nc.scalar.dma_start(ovl_out[:, t * G : (t + 1) * G, :], oo[:])

            # ---- node onsite energies via exact cubic in atom type ----
            at = one.tile([128, KN], i32)
            nc.sync.dma_start(at[:], atype[:, :])
            af = one.tile([128, KN], f32)
            nc.vector.tensor_copy(out=af[:], in_=at[:])
            nod = one.tile([128, KN, 4], f32)
            u = one.tile([128, KN], f32)
            for j in range(4):
                c3, c2, c1, c0 = (float(coefs[0, j]), float(coefs[1, j]),
                                  float(coefs[2, j]), float(coefs[3, j]))
                nc.vector.tensor_scalar(out=u[:], in0=af[:], scalar1=c3, scalar2=c2,
                                        op0=mybir.AluOpType.mult, op1=mybir.AluOpType.add)
                nc.vector.tensor_tensor(out=u[:], in0=u[:], in1=af[:],
                                        op=mybir.AluOpType.mult)
                nc.vector.tensor_scalar(out=u[:], in0=u[:], scalar1=c1, scalar2=None,
                                        op0=mybir.AluOpType.add)
                nc.vector.tensor_tensor(out=u[:], in0=u[:], in1=af[:],
                                        op=mybir.AluOpType.mult)
                nc.vector.tensor_scalar(out=nod[:, :, j], in0=u[:], scalar1=c0,
                                        scalar2=None, op0=mybir.AluOpType.add)
            nc.sync.dma_start(node_out[:, :, :], nod[:])

    nc.compile()
    return nc


def kernel(rij, edge_type, atom_type, hopping_tables, overlap_tables, onsiteE,
           trace=False):
    global LAST_RESULTS
    rij = np.asarray(rij, np.float32)
    edge_type = np.asarray(edge_type, np.int32)
    atom_type = np.asarray(atom_type, np.int32)
    hopping_tables = np.asarray(hopping_tables, np.float32)
    overlap_tables = np.asarray(overlap_tables, np.float32)
    onsiteE = np.asarray(onsiteE, np.float32)

    ptab = _build_pair_table(hopping_tables, overlap_tables)
    coefs = _node_poly_coeffs(onsiteE)
    nc = _build_program(coefs)

    in_maps = []
    for c in range(NCORES):
        r = np.full(E_PAD, 2.0, np.float32)
        e = np.zeros(E_PAD, np.int32)
        r[:E_SHARD] = rij[c * E_SHARD : (c + 1) * E_SHARD]
        e[:E_SHARD] = edge_type[c * E_SHARD : (c + 1) * E_SHARD]
        a = np.zeros(N_PAD, np.int32)
        a[:N_SHARD] = atom_type[c * N_SHARD : (c + 1) * N_SHARD]
        in_maps.append({
            "ptab": ptab,
            "rij16": np.ascontiguousarray(r.reshape(DT, 16).T),
            "et16": np.ascontiguousarray(e.reshape(DT, 16).T),
            "rij128": np.ascontiguousarray(r.reshape(GT, 128).T),
            "atype": np.ascontiguousarray(a.reshape(128, KN)),
        })

    kwargs = {}
    if trace:
        kwargs = {"trace": True, "trace_cores": [0]}
    res = run_bass_kernel_spmd(nc, in_maps, core_ids=list(range(NCORES)), **kwargs)
    LAST_RESULTS = res

    edge_features = np.empty((E_TOTAL, M), np.float32)
    edge_overlap = np.empty((E_TOTAL, M), np.float32)
    node_features = np.empty((N_TOTAL, N_ONSITE), np.float32)
    for c in range(NCORES):
        out = res.results[c]
        hop = out["hop_out"].reshape(128, T, G, M).transpose(1, 2, 0, 3).reshape(E_PAD, M)
        ovl = out["ovl_out"].reshape(128, T, G, M).transpose(1, 2, 0, 3).reshape(E_PAD, M)
        edge_features[c * E_SHARD : (c + 1) * E_SHARD] = hop[:E_SHARD]
        edge_overlap[c * E_SHARD : (c + 1) * E_SHARD] = ovl[:E_SHARD]
        node_features[c * N_SHARD : (c + 1) * N_SHARD] = (
            out["node_out"].reshape(N_PAD, N_ONSITE)[:N_SHARD])
    return edge_features, edge_overlap, node_features


# revision 5
# speedup vs baseline: 1.4685x; 1.4685x over previous
"""Trainium2 Bass kernel for DFTB Slater-Koster table interpolation.

Problem (see reference): for 2M edges, linearly interpolate 16 hopping and 16
overlap integrals from per-bond-type tables [10, 16, 512] at distance rij;
for 500K nodes, look up onsite energies [4, 4] by atom type.

Strategy (8 NeuronCores, data-parallel over edges/nodes):
  - Host packs a "pair table" [5120, 64] f32 where row (b*512+g) holds
    [hop[b,:,g], ovl[b,:,g], hop[b,:,g+1]-hop[b,:,g], ovl[b,:,g+1]-ovl[b,:,g]]
    so one 256B SWDGE dma_gather descriptor per edge fetches everything needed.
  - Device computes i0 = floor((rij-X0)/DX) (exact floor, robust to the
    engine's f32->int rounding mode) and the combined table row index
    idx = etype*512 + i0 as int16, gathers 256B/edge with dma_gather, then
    interpolates y0 + frac*delta on the vector engine with a per-edge
    broadcast of frac.
  - Edges are processed in gather tiles of NI=4096, issued round-robin on all
    4 SWDGE queues: each queue's Q7 context generates descriptors in parallel
    (~2ns/idx aggregate vs ~8ns/idx on one queue), and NI/16=256 descriptors
    per engine leaves ring space (1024/engine) for several tiles in flight.
    dma_gather places edge j of a tile at SBUF (partition j%128, group
    j//128) and reads its index from (partition j%16, column j//16); the host
    pre-packs rij into both wrap-16 (index path) and wrap-128 (frac path)
    layouts so every device DMA is a plain linear access pattern, and outputs
    are written partition-major so write descriptors are 4KB-contiguous.
  - Node onsite lookup: with only 4 atom types, evaluate the cubic polynomial
    through the 4 table values (exact interpolation at a=0..3) instead of a
    gather.
"""
import numpy as np

import concourse.bass as bass
import concourse.bacc as bacc
import concourse.mybir as mybir
import concourse.tile as tile
from concourse import library_config
from concourse.bass_utils import run_bass_kernel_spmd

# ---- problem constants (hardcoded; must match the reference) ----
NGRID = 512
X0 = 1.0
XMAX = 10.0
DX = (XMAX - X0) / (NGRID - 1)
NB = 10
M = 16
NROWS = NB * NGRID          # 5120 pair-table rows
ES = 4 * M                  # 64 f32 = 256B per row
N_ATOM_TYPES = 4
N_ONSITE = 4

NCORES = 8
E_TOTAL = 2_000_000
N_TOTAL = 500_000

# per-core tiling
E_SHARD = E_TOTAL // NCORES          # 250_000
NI = 4096                            # edges per gather tile
T = 64                               # gather tiles per core
E_PAD = T * NI                       # 262_144
G = NI // 128                        # 32 groups per tile
D = NI // 16                         # 256 idx cols per tile
GT = T * G                           # 2048
DT = T * D                           # 16384
N_SHARD = N_TOTAL // NCORES          # 62_500
KN = 512                             # node cols per partition
N_PAD = 128 * KN                     # 65_536

LAST_RESULTS = None  # BassKernelResults of the most recent run (for profiling)


def _build_pair_table(hop: np.ndarray, ovl: np.ndarray) -> np.ndarray:
    ptab = np.empty((NB, NGRID, ES), np.float32)
    h = np.ascontiguousarray(hop.transpose(0, 2, 1))  # [NB, G, M]
    o = np.ascontiguousarray(ovl.transpose(0, 2, 1))
    ptab[:, :, 0:16] = h
    ptab[:, :, 16:32] = o
    ptab[:, :-1, 32:48] = h[:, 1:] - h[:, :-1]
    ptab[:, -1, 32:48] = 0.0
    ptab[:, :-1, 48:64] = o[:, 1:] - o[:, :-1]
    ptab[:, -1, 48:64] = 0.0
    return ptab.reshape(NROWS, ES)


def _node_poly_coeffs(onsiteE: np.ndarray) -> np.ndarray:
    """Cubic through onsiteE[a, j] at a = 0..3 (exact). [4 coefs, 4 feats]."""
    A = np.vander(np.arange(4.0), 4, increasing=False)
    return np.linalg.solve(A.astype(np.float64), onsiteE.astype(np.float64))


def _emit_floor(nc, pool, src_f32, parts, width, tagp):
    """Exact floor of src (values >= -0.5) as an f32 tile.
    i0f = float(int_cast(src)); if i0f > src: i0f -= 1. Correct whether the
    engine's f32->int cast truncates or rounds to nearest."""
    f32, i32 = mybir.dt.float32, mybir.dt.int32
    ii = pool.tile([parts, width], i32, tag=f"{tagp}_ii")
    nc.vector.tensor_copy(out=ii[:], in_=src_f32)
    ff = pool.tile([parts, width], f32, tag=f"{tagp}_ff")
    nc.vector.tensor_copy(out=ff[:], in_=ii[:])
    gt = pool.tile([parts, width], f32, tag=f"{tagp}_gt")
    nc.vector.tensor_tensor(out=gt[:], in0=ff[:], in1=src_f32,
                            op=mybir.AluOpType.is_gt)
    nc.vector.tensor_tensor(out=ff[:], in0=ff[:], in1=gt[:],
                            op=mybir.AluOpType.subtract)
    return ff


def _build_program(coefs: np.ndarray):
    nc = bacc.Bacc("TRN2", target_bir_lowering=False, debug=False,
                   num_swdge_queues=4)
    f32, i32, i16 = mybir.dt.float32, mybir.dt.int32, mybir.dt.int16

    ptab = nc.dram_tensor("ptab", [NROWS, ES], f32, kind="ExternalInput")
    rij16 = nc.dram_tensor("rij16", [16, DT], f32, kind="ExternalInput")
    et16 = nc.dram_tensor("et16", [16, DT], i32, kind="ExternalInput")
    rij128 = nc.dram_tensor("rij128", [128, GT], f32, kind="ExternalInput")
    atype = nc.dram_tensor("atype", [128, KN], i32, kind="ExternalInput")
    hop_out = nc.dram_tensor("hop_out", [128, GT, M], f32, kind="ExternalOutput")
    ovl_out = nc.dram_tensor("ovl_out", [128, GT, M], f32, kind="ExternalOutput")
    node_out = nc.dram_tensor("node_out", [128, KN, 4], f32, kind="ExternalOutput")
    idx_dram = nc.dram_tensor("idx_scratch", [T, 16, D], i16, kind="Internal")

    inv_dx = 1.0 / DX

    with tile.TileContext(nc) as tc:
        with (
            tc.tile_pool(name="io", bufs=6) as io,
            tc.tile_pool(name="gat", bufs=8) as gat,
            tc.tile_pool(name="sm", bufs=4) as sm,
            tc.tile_pool(name="one", bufs=1) as one,
        ):
            nc.gpsimd.load_library(library_config.mlp)

            for t in range(T):
                # ---- index path, wrap-16 layout [16, D] ----
                rv = sm.tile([16, D], f32, tag="rv")
                nc.sync.dma_start(rv[:], rij16[:, t * D : (t + 1) * D])
                ev = sm.tile([16, D], i32, tag="ev")
                nc.sync.dma_start(ev[:], et16[:, t * D : (t + 1) * D])
                # t = (rij - X0) / DX   (in-place on rv)
                nc.vector.tensor_scalar(
                    out=rv[:], in0=rv[:], scalar1=X0, scalar2=inv_dx,
                    op0=mybir.AluOpType.subtract, op1=mybir.AluOpType.mult)
                i0f = _emit_floor(nc, sm, rv[:], 16, D, "ix")
                i0i = sm.tile([16, D], i32, tag="i0i")
                nc.vector.tensor_copy(out=i0i[:], in_=i0f[:])  # integral: exact
                # idx = (etype << 9) + i0   (in-place on ev)
                nc.vector.tensor_scalar(
                    out=ev[:], in0=ev[:], scalar1=9, scalar2=None,
                    op0=mybir.AluOpType.logical_shift_left)
                nc.vector.tensor_tensor(out=ev[:], in0=ev[:], in1=i0i[:],
                                        op=mybir.AluOpType.add)
                ix16 = sm.tile([16, D], i16, tag="ix16")
                nc.vector.tensor_copy(out=ix16[:], in_=ev[:])
                # replicate [16, D] -> [128, D] via DRAM roundtrip (the gather
                # engine reads indices from all 8 partition groups)
                nc.sync.dma_start(idx_dram[t], ix16[:])
                rep = io.tile([128, D], i16, tag="rep")
                rep_src = bass.AP(idx_dram, t * 16 * D, [[0, 8], [D, 16], [1, D]])
                nc.sync.dma_start(rep[:], rep_src)
                # ---- frac path, wrap-128 layout [128, G] ----
                rf = sm.tile([128, G], f32, tag="rf")
                nc.scalar.dma_start(rf[:], rij128[:, t * G : (t + 1) * G])
                nc.vector.tensor_scalar(
                    out=rf[:], in0=rf[:], scalar1=X0, scalar2=inv_dx,
                    op0=mybir.AluOpType.subtract, op1=mybir.AluOpType.mult)
                f0f = _emit_floor(nc, sm, rf[:], 128, G, "fr")
                frac = sm.tile([128, G], f32, tag="frac")
                nc.vector.tensor_tensor(out=frac[:], in0=rf[:], in1=f0f[:],
                                        op=mybir.AluOpType.subtract)
                # ---- gather: 256B per edge ----
                dst = gat.tile([128, G, ES], f32, tag="dst")
                nc.gpsimd.dma_gather(dst[:], ptab[:, :], rep[:], NI, NI, ES,
                                     single_packet=False, queue_num=t % 4)
                # ---- interpolate: out = y0 + frac * delta ----
                fb = frac[:].rearrange("p (g o) -> p g o", o=1).to_broadcast([128, G, M])
                oh = io.tile([128, G, M], f32, tag="oh")
                nc.vector.tensor_tensor(out=oh[:], in0=dst[:, :, 32:48], in1=fb,
                                        op=mybir.AluOpType.mult)
                nc.vector.tensor_tensor(out=oh[:], in0=oh[:], in1=dst[:, :, 0:16],
                                        op=mybir.AluOpType.add)
                oo = io.tile([128, G, M], f32, tag="oo")
                nc.vector.tensor_tensor(out=oo[:], in0=dst[:, :, 48:64], in1=fb,
                                        op=mybir.AluOpType.mult)
                nc.vector.tensor_tensor(out=oo[:], in0=oo[:], in1=dst[:, :, 16:32],
                                        op=mybir.AluOpType.add)
                nc.scalar.dma_start(hop_out[:, t * G : (t + 1) * G, :], oh[:])
                nc.scalar.dma_start(ovl_out[:, t * G : (t + 1) * G, :], oo[:])

            # ---- node onsite energies via exact cubic in atom type ----
            at = one.tile([128, KN], i32)
            nc.sync.dma_start(at[:], atype[:, :])
            af = one.tile([128, KN], f32)
            nc.vector.tensor_copy(out=af[:], in_=at[:])
            nod = one.tile([128, KN, 4], f32)
            u = one.tile([128, KN], f32)
            for j in range(4):
                c3, c2, c1, c0 = (float(coefs[0, j]), float(coefs[1, j]),
                                  float(coefs[2, j]), float(coefs[3, j]))
                nc.vector.tensor_scalar(out=u[:], in0=af[:], scalar1=c3, scalar2=c2,
                                        op0=mybir.AluOpType.mult, op1=mybir.AluOpType.add)
                nc.vector.tensor_tensor(out=u[:], in0=u[:], in1=af[:],
                                        op=mybir.AluOpType.mult)
                nc.vector.tensor_scalar(out=u[:], in0=u[:], scalar1=c1, scalar2=None,
                                        op0=mybir.AluOpType.add)
                nc.vector.tensor_tensor(out=u[:], in0=u[:], in1=af[:],
                                        op=mybir.AluOpType.mult)
                nc.vector.tensor_scalar(out=nod[:, :, j], in0=u[:], scalar1=c0,
                                        scalar2=None, op0=mybir.AluOpType.add)
            nc.sync.dma_start(node_out[:, :, :], nod[:])

    nc.compile()
    return nc


def kernel(rij, edge_type, atom_type, hopping_tables, overlap_tables, onsiteE,
           trace=False):
    global LAST_RESULTS
    rij = np.asarray(rij, np.float32)
    edge_type = np.asarray(edge_type, np.int32)
    atom_type = np.asarray(atom_type, np.int32)
    hopping_tables = np.asarray(hopping_tables, np.float32)
    overlap_tables = np.asarray(overlap_tables, np.float32)
    onsiteE = np.asarray(onsiteE, np.float32)

    ptab = _build_pair_table(hopping_tables, overlap_tables)
    coefs = _node_poly_coeffs(onsiteE)
    nc = _build_program(coefs)

    in_maps = []
    for c in range(NCORES):
        r = np.full(E_PAD, 2.0, np.float32)
        e = np.zeros(E_PAD, np.int32)
        r[:E_SHARD] = rij[c * E_SHARD : (c + 1) * E_SHARD]
        e[:E_SHARD] = edge_type[c * E_SHARD : (c + 1) * E_SHARD]
        a = np.zeros(N_PAD, np.int32)
        a[:N_SHARD] = atom_type[c * N_SHARD : (c + 1) * N_SHARD]
        in_maps.append({
            "ptab": ptab,
            "rij16": np.ascontiguousarray(r.reshape(DT, 16).T),
            "et16": np.ascontiguousarray(e.reshape(DT, 16).T),
            "rij128": np.ascontiguousarray(r.reshape(GT, 128).T),
            "atype": np.ascontiguousarray(a.reshape(128, KN)),
        })

    kwargs = {}
    if trace:
        kwargs = {"trace": True, "trace_cores": [0]}
    res = run_bass_kernel_spmd(nc, in_maps, core_ids=list(range(NCORES)), **kwargs)
    LAST_RESULTS = res

    edge_features = np.empty((E_TOTAL, M), np.float32)
    edge_overlap = np.empty((E_TOTAL, M), np.float32)
    node_features = np.empty((N_TOTAL, N_ONSITE), np.float32)
    for c in range(NCORES):
        out = res.results[c]
        hop = out["hop_out"].reshape(128, T, G, M).transpose(1, 2, 0, 3).reshape(E_PAD, M)
        ovl = out["ovl_out"].reshape(128, T, G, M).transpose(1, 2, 0, 3).reshape(E_PAD, M)
        edge_features[c * E_SHARD : (c + 1) * E_SHARD] = hop[:E_SHARD]
        edge_overlap[c * E_SHARD : (c + 1) * E_SHARD] = ovl[:E_SHARD]
        node_features[c * N_SHARD : (c + 1) * N_SHARD] = (
            out["node_out"].reshape(N_PAD, N_ONSITE)[:N_SHARD])
    return edge_features, edge_overlap, node_features
